# revision 4
# baseline (speedup 1.0000x reference)
"""Trainium2 Bass kernel for NoMlpGpt2Attention (B=2, S=2048, E=1024, H=16, D=64).

Sharding: 8 cores = data-parallel over batch (2) x tensor-parallel over head
groups (4 groups of 4 heads). Each core computes q/k for its 4 heads, the full
v projection, the gated-V FFN for its heads' columns, causal attention for its
heads, and a partial output projection. Host sums the 4 partials per batch and
adds b_proj.

Device math (per core, fixed b, 4 local heads):
  hT = hidden[b].T                        [E, S]   (host-transposed input)
  qkT = Wqk.T @ hT (+bias, q scaled)      [512, S] rows = (q|k, head, d)
  vT  = Wv.T @ hT (+bias)                 [E, S]
  gate/ff = vT.T @ Wg|Wf  -> vg = relu(gate)*ff   [S, 260] (65 cols/head, last=1)
  scoresT[j,i] = kT_j . qT_i  (fp32r), exp, tril mask on diagonal blocks
  attnT[d,i] (+ sum row via ones column) = vg_aug.T @ expT   (causal j-tiles)
  attn_norm = attnT[:64] * (1/attnT[64])  broadcast over partitions
  out_partial = attn_normT.T @ Wp         [S, E]
All matmuls run as float32r (tf32-class rounding, fp32 accumulate).
"""

import math
import os
import sys

import numpy as np

for _p in ("/opt/trn_rl_repo", "/root/.axon_site/_ro/trn_rl_repo"):
    if os.path.isdir(_p) and _p not in sys.path:
        sys.path.insert(0, _p)

B, S, E, H, D = 2, 2048, 1024, 16, 64
HL = 4            # heads per core
ET = E // 128     # 8 contraction tiles
SQ = 512          # sequence quarter processed per projection pass
NQ = S // SQ
JT = S // 128     # 16 key tiles

_CACHE = {}


def _build(scale: float, has_qkv_bias: bool):
    import concourse.bass as bass  # noqa: F401
    import concourse.mybir as mybir
    import concourse.tile as tile
    from concourse import bacc

    dt = mybir.dt
    F32 = dt.float32
    F32R = dt.float32r
    AF = mybir.ActivationFunctionType
    ALU = mybir.AluOpType

    nc = bacc.Bacc("TRN2", target_bir_lowering=False, debug=False)

    hT_d = nc.dram_tensor("hT", [E, S], F32, kind="ExternalInput")
    wqk_d = nc.dram_tensor("wqk", [E, 2 * HL * D], F32, kind="ExternalInput")
    wv_d = nc.dram_tensor("wv", [E, E], F32, kind="ExternalInput")
    wg_d = nc.dram_tensor("wg", [E, HL * D], F32, kind="ExternalInput")
    wf_d = nc.dram_tensor("wf", [E, HL * D], F32, kind="ExternalInput")
    wp_d = nc.dram_tensor("wp", [HL * D, E], F32, kind="ExternalInput")
    tril_d = nc.dram_tensor("tril", [128, 128], F32, kind="ExternalInput")
    if has_qkv_bias:
        bqk_d = nc.dram_tensor("bqk", [2 * HL * D, 1], F32, kind="ExternalInput")
        bv_d = nc.dram_tensor("bv", [E, 1], F32, kind="ExternalInput")
    out_d = nc.dram_tensor("out", [S, E], F32, kind="ExternalOutput")

    def r32(ap):
        return ap.bitcast(F32R)

    with tile.TileContext(nc) as tc:
        with tc.tile_pool(name="persist", bufs=1) as persist:
            # ---- long-lived tensors (~74 KB/partition) ----
            wp_t = [persist.tile([128, 1024], F32R, name=f"wp{k}", tag=f"wp{k}")
                    for k in range(2)]
            tril_t = persist.tile([128, 128], F32R, name="tril", tag="tril")
            for k in range(2):
                nc.sync.dma_start(wp_t[k][:], r32(wp_d.ap()[k * 128:(k + 1) * 128, :]))
            nc.sync.dma_start(tril_t[:], r32(tril_d.ap()))
            onecol_t = persist.tile([128, 1], F32, name="onecol", tag="onecol")
            nc.vector.memset(onecol_t[:], 1.0)
            qkT_t = [persist.tile([128, S], F32R, name=f"qkT{m}", tag=f"qkT{m}")
                     for m in range(4)]
            vg_t = [persist.tile([128, HL * 65], F32R, name=f"vg{s}", tag=f"vg{s}")
                    for s in range(S // 128)]
            attn_all = [persist.tile([128, S], F32R, name=f"attnall{k}", tag=f"attnall{k}")
                        for k in range(2)]

            # ============ phase 1: projections + gated-V, by S-quarter ============
            with (
                tc.tile_pool(name="p1", bufs=1) as p1,
                tc.tile_pool(name="ps1", bufs=1, space="PSUM") as ps1,
            ):
                wqk_t = [p1.tile([128, 512], F32R, name=f"wqk{e}", tag=f"wqk{e}")
                         for e in range(ET)]
                wv_t = [p1.tile([128, 1024], F32R, name=f"wv{e}", tag=f"wv{e}")
                        for e in range(ET)]
                wg_t = [p1.tile([128, 256], F32R, name=f"wg{e}", tag=f"wg{e}")
                        for e in range(ET)]
                wf_t = [p1.tile([128, 256], F32R, name=f"wf{e}", tag=f"wf{e}")
                        for e in range(ET)]
                for e in range(ET):
                    rows = slice(e * 128, (e + 1) * 128)
                    nc.sync.dma_start(wqk_t[e][:], r32(wqk_d.ap()[rows, :]))
                    nc.sync.dma_start(wv_t[e][:], r32(wv_d.ap()[rows, :]))
                    nc.sync.dma_start(wg_t[e][:], r32(wg_d.ap()[rows, :]))
                    nc.sync.dma_start(wf_t[e][:], r32(wf_d.ap()[rows, :]))
                if has_qkv_bias:
                    bqk_t = [p1.tile([128, 1], F32, name=f"bqk{m}", tag=f"bqk{m}")
                             for m in range(4)]
                    bv_t = [p1.tile([128, 1], F32, name=f"bv{e}", tag=f"bv{e}")
                            for e in range(ET)]
                    for m in range(4):
                        nc.sync.dma_start(bqk_t[m][:], bqk_d.ap()[m * 128:(m + 1) * 128, :])
                    for e in range(ET):
                        nc.sync.dma_start(bv_t[e][:], bv_d.ap()[e * 128:(e + 1) * 128, :])

                for q in range(NQ):
                    cols = slice(q * SQ, (q + 1) * SQ)
                    hT_t = []
                    for e in range(ET):
                        h_tile = p1.tile([128, SQ], F32R, name=f"hT{e}_{q}",
                                         tag=f"hT{e}", bufs=2)
                        nc.sync.dma_start(h_tile[:], r32(hT_d.ap()[e * 128:(e + 1) * 128, cols]))
                        hT_t.append(h_tile)

                    # q/k projection -> qkT rows (q heads scaled by `scale`)
                    for m in range(4):
                        ps = ps1.tile([128, SQ], F32, name=f"psqk{m}_{q}", tag="ps_qk", bufs=2)
                        for e in range(ET):
                            nc.tensor.matmul(ps[:], wqk_t[e][:, m * 128:(m + 1) * 128],
                                             hT_t[e][:], start=(e == 0), stop=(e == ET - 1))
                        dst = qkT_t[m][:, cols]
                        if has_qkv_bias:
                            nc.vector.tensor_scalar(dst, ps[:], bqk_t[m][:],
                                                    scale if m < 2 else 1.0,
                                                    ALU.add, ALU.mult)
                        elif m < 2:
                            nc.scalar.mul(dst, ps[:], scale)
                        else:
                            nc.scalar.copy(dst, ps[:])

                    # v projection -> vT (quarter-local)
                    vT_t = []
                    for m in range(ET):
                        ps = ps1.tile([128, SQ], F32, name=f"psv{m}_{q}", tag="ps_v", bufs=2)
                        for e in range(ET):
                            nc.tensor.matmul(ps[:], wv_t[e][:, m * 128:(m + 1) * 128],
                                             hT_t[e][:], start=(e == 0), stop=(e == ET - 1))
                        v_tile = p1.tile([128, SQ], F32R, name=f"vT{m}_{q}", tag=f"vT{m}")
                        if has_qkv_bias:
                            nc.vector.tensor_scalar(v_tile[:], ps[:], bv_t[m][:], 1.0,
                                                    ALU.add, ALU.mult)
                        else:
                            nc.scalar.copy(v_tile[:], ps[:])
                        vT_t.append(v_tile)

                    # gated V ffn -> vg (65 cols per head, last col = 1)
                    for sb in range(SQ // 128):
                        st = q * (SQ // 128) + sb
                        scol = slice(sb * 128, (sb + 1) * 128)
                        psg = ps1.tile([128, 256], F32, name=f"psg{st}", tag="ps_g", bufs=2)
                        psf = ps1.tile([128, 256], F32, name=f"psf{st}", tag="ps_f", bufs=2)
                        for e in range(ET):
                            nc.tensor.matmul(psg[:], vT_t[e][:, scol], wg_t[e][:],
                                             start=(e == 0), stop=(e == ET - 1))
                        for e in range(ET):
                            nc.tensor.matmul(psf[:], vT_t[e][:, scol], wf_t[e][:],
                                             start=(e == 0), stop=(e == ET - 1))
                        relu_t = p1.tile([128, 256], F32, name=f"relu{st}", tag="relu", bufs=2)
                        nc.vector.tensor_relu(relu_t[:], psg[:])
                        vg3 = vg_t[st].rearrange("p (h x) -> p h x", x=65)
                        nc.vector.tensor_mul(vg3[:, :, 0:64],
                                             relu_t[:].rearrange("p (h x) -> p h x", x=64),
                                             psf[:].rearrange("p (h x) -> p h x", x=64))
                        for hh in range(HL):
                            nc.vector.tensor_copy(vg_t[st][:, hh * 65 + 64:hh * 65 + 65],
                                                  onecol_t[:])

            # ================= phase 2: causal attention per head =================
            with (
                tc.tile_pool(name="p2", bufs=1) as p2,
                tc.tile_pool(name="ps2", bufs=1, space="PSUM") as ps2,
            ):
                for h in range(HL):
                    off = (h % 2) * 64
                    qt = qkT_t[h // 2]
                    kt_ = qkT_t[2 + h // 2]
                    attn_ps = ps2.tile([65, S], F32, name=f"attn{h}", tag="attn", bufs=1)
                    for jt in range(JT):
                        j0 = jt * 128
                        ni = S - j0
                        for t in range(math.ceil(ni / 1024)):
                            a0 = j0 + t * 1024
                            tw = min(1024, S - a0)
                            sps = ps2.tile([128, tw], F32, name=f"sc{h}_{jt}_{t}",
                                           tag="scores", bufs=2)
                            for c in range(math.ceil(tw / 512)):
                                cw = min(512, tw - c * 512)
                                nc.tensor.matmul(sps[:, c * 512:c * 512 + cw],
                                                 kt_[off:off + 64, j0:j0 + 128],
                                                 qt[off:off + 64, a0 + c * 512:a0 + c * 512 + cw],
                                                 start=True, stop=True)
                            expt = p2.tile([128, tw], F32R, name=f"exp{h}_{jt}_{t}",
                                           tag="exp", bufs=3)
                            nc.scalar.activation(expt[:], sps[:], AF.Exp)
                            if t == 0:
                                nc.vector.tensor_mul(expt[:, 0:128], expt[:, 0:128], tril_t[:])
                            pts = sorted({a0, a0 + tw} | {bb * 512 for bb in range(1, 4)
                                                          if a0 < bb * 512 < a0 + tw})
                            for p0, p1_ in zip(pts, pts[1:]):
                                bank = p0 // 512
                                nc.tensor.matmul(attn_ps[:, p0:p1_],
                                                 vg_t[jt][:, h * 65:(h + 1) * 65],
                                                 expt[:, p0 - a0:p1_ - a0],
                                                 start=(jt == 0),
                                                 stop=(jt == min(JT - 1, 4 * bank + 3)))
                    # normalize rows 0..63 by the sum row (64), broadcast over partitions
                    recip_t = p2.tile([1, S], F32, name=f"recip{h}", tag="recip", bufs=1)
                    nc.vector.reciprocal(recip_t[:], attn_ps[64:65, :])
                    bcast_t = p2.tile([64, S], F32, name=f"bcast{h}", tag="bcast", bufs=2)
                    nc.gpsimd.partition_broadcast(bcast_t[:], recip_t[:])
                    nc.vector.tensor_mul(attn_all[h // 2][off:off + 64, :],
                                         attn_ps[0:64, :], bcast_t[:])

            # ================= phase 3: output projection =================
            with (
                tc.tile_pool(name="p3", bufs=1) as p3,
                tc.tile_pool(name="ps3", bufs=1, space="PSUM") as ps3,
            ):
                for sb in range(S // 128):
                    ps_o = ps3.tile([128, 1024], F32, name=f"pso{sb}", tag="ps_o", bufs=2)
                    for k in range(2):
                        for c in range(2):
                            nc.tensor.matmul(ps_o[:, c * 512:(c + 1) * 512],
                                             attn_all[k][:, sb * 128:(sb + 1) * 128],
                                             wp_t[k][:, c * 512:(c + 1) * 512],
                                             start=(k == 0), stop=(k == 1))
                    outt = p3.tile([128, 1024], F32, name=f"out{sb}", tag="outsb", bufs=3)
                    nc.vector.tensor_copy(outt[:], ps_o[:])
                    nc.sync.dma_start(out_d.ap()[sb * 128:(sb + 1) * 128, :], outt[:])

    nc.compile()
    return nc


def _get_nc(scale: float, has_qkv_bias: bool):
    key = (round(scale, 12), has_qkv_bias)
    if key not in _CACHE:
        _CACHE[key] = _build(scale, has_qkv_bias)
    return _CACHE[key]


def _shard_inputs(hidden, W_attn, b_attn, W_proj, W_v_ff, W_v_gate):
    tril = np.triu(np.ones((128, 128), dtype=np.float32))  # [j, i]: 1 iff j <= i
    wv = np.ascontiguousarray(W_attn[:, 2].reshape(E, E))
    has_bias = bool(np.any(b_attn))
    maps = []
    for core in range(8):
        b = core // 4
        hg = core % 4
        hs = slice(hg * HL, (hg + 1) * HL)
        cols = slice(hg * HL * D, (hg + 1) * HL * D)
        m = {
            "hT": np.ascontiguousarray(hidden[b].T),
            "wqk": np.ascontiguousarray(W_attn[:, 0:2, hs, :].reshape(E, 2 * HL * D)),
            "wv": wv,
            "wg": np.ascontiguousarray(W_v_gate[:, cols]),
            "wf": np.ascontiguousarray(W_v_ff[:, cols]),
            "wp": np.ascontiguousarray(W_proj[hs].reshape(HL * D, E)),
            "tril": tril,
        }
        if has_bias:
            m["bqk"] = np.ascontiguousarray(b_attn[0:2, hs].reshape(2 * HL * D, 1))
            m["bv"] = np.ascontiguousarray(b_attn[2].reshape(E, 1))
        maps.append(m)
    return maps, has_bias


def _reference_numpy(hidden_states, mask, layer_idx, W_attn, b_attn, W_proj,
                     b_proj, W_v_ff, W_v_gate):
    """Exact fallback for non-causal masks."""
    h = hidden_states.astype(np.float64)
    qkv = np.einsum("bse,eqhd->bqshd", h, W_attn.astype(np.float64)) \
        + b_attn.astype(np.float64)[:, None, :, :]
    q, k, v = qkv[:, 0], qkv[:, 1], qkv[:, 2]
    q = q * (D ** -0.5 / (float(layer_idx) + 1.0))
    scores = np.einsum("bshd,bthd->bhst", q, k)
    scores = scores + (1.0 - mask.astype(np.float64)) * -1e9
    scores -= scores.max(axis=-1, keepdims=True)
    w = np.exp(scores)
    w /= w.sum(axis=-1, keepdims=True)
    v_flat = v.reshape(B, S, E)
    v_gated = (np.maximum(v_flat @ W_v_gate.astype(np.float64), 0.0) *
               (v_flat @ W_v_ff.astype(np.float64))).reshape(B, S, H, D)
    attn = np.einsum("bhst,bthd->bshd", w, v_gated)
    out = np.einsum("bshd,hde->bse", attn, W_proj.astype(np.float64)) \
        + b_proj.astype(np.float64)
    return out.astype(np.float32)


def kernel(hidden_states, mask, layer_idx, W_attn, b_attn, W_proj, b_proj,
           W_v_ff, W_v_gate):
    from concourse import bass_utils

    hidden_states = np.asarray(hidden_states, dtype=np.float32)
    mask = np.asarray(mask, dtype=np.float32)
    W_attn = np.asarray(W_attn, dtype=np.float32)
    b_attn = np.asarray(b_attn, dtype=np.float32)
    W_proj = np.asarray(W_proj, dtype=np.float32)
    b_proj = np.asarray(b_proj, dtype=np.float32)
    W_v_ff = np.asarray(W_v_ff, dtype=np.float32)
    W_v_gate = np.asarray(W_v_gate, dtype=np.float32)
    lidx = int(np.asarray(layer_idx))

    if not np.array_equal(mask, np.tril(np.ones((S, S), dtype=np.float32))):
        return _reference_numpy(hidden_states, mask, lidx, W_attn, b_attn,
                                W_proj, b_proj, W_v_ff, W_v_gate)

    scale = float(D ** -0.5 / (lidx + 1.0))
    in_maps, has_bias = _shard_inputs(hidden_states, W_attn, b_attn, W_proj,
                                      W_v_ff, W_v_gate)
    nc = _get_nc(scale, has_bias)
    res = bass_utils.run_bass_kernel_spmd(nc, in_maps, core_ids=list(range(8)))
    out = np.empty((B, S, E), dtype=np.float32)
    for b in range(B):
        acc = res.results[4 * b]["out"].astype(np.float32).copy()
        for hg in range(1, 4):
            acc += res.results[4 * b + hg]["out"]
        out[b] = acc + b_proj
    return out
